# revision 8
# baseline (speedup 1.0000x reference)
"""LIF layer (T=64, B=128, 2048->2048) on 8 trn2 NeuronCores.

Sharding: 4-way over out_dim x 2-way over batch. Each core owns
O_loc=512 output channels (4 chunks of 128) and B_loc=64 batch rows.
The 512-wide moving streams let the f32r weight load (~195ns, no FWL
for 4-byte weights) hide behind each 213ns matmul, keeping the PE at
the f32r roofline (~116us/core).

GEMM: single-pass float32r (PE rounds inputs to ~fp22 with RNE;
1 cycle/row). Host pre-rounds inputs RNE to 11 mantissa bits (measured
bit-identical to HW rounding -> documents the precision contract).
Per 8-timestep block, psum holds [128, 4 chunks, 512 cols]; chunk <->
bank, double-buffered across blocks.

Scan: z-space reformulation removes the per-step decay multiply. Host
prescales x columns of step tau (within a block) by s_tau = d^-(tau+1);
bias and the threshold shift enter through a rank-1 17th matmul (bias
row). With the sign-flipped state ybar = -d^-tau*(mem-1), one LIF step
is 3 DVE ops:
    A: ybar -= G               (psum read)
    B: spk = ybar < 0          (exact {0,1}, written bf16 for cheap DMA)
    C: ybar = spk*s_tau + ybar (fused scalar_tensor_tensor)
and ybar *= d^8 once per block. Spikes DMA out per step; host casts
bf16 {0,1} back to fp32 exactly.
"""

import math

import numpy as np

import concourse.bacc as bacc
import concourse.bass as bass
import concourse.mybir as mybir
import concourse.tile as tile
from concourse import bass_utils

# Problem constants (hardcoded per contract)
T, B, I, O = 64, 128, 2048, 2048
N_CORES = 8
OC_SHARD, BC_SHARD = 4, 2          # out_dim x batch sharding grid
O_LOC = O // OC_SHARD              # 512 channels per core
B_LOC = B // BC_SHARD              # 64 batch rows per core
N_CHUNK = O_LOC // 128             # 4 stationary chunks
KT = I // 128                      # 16 k-tiles
STEPS_PER_BLK = 8                  # timesteps per psum block
N_BLK = T // STEPS_PER_BLK         # 8 blocks
COLS = STEPS_PER_BLK * B_LOC       # 512 moving columns per block
CHUNKS_PER_BANK = max(1, 512 // COLS)
TAU_C, THR = 2.0, 1.0
DECAY = math.exp(-1.0 / TAU_C)
SCALES = [DECAY ** -(t + 1) for t in range(STEPS_PER_BLK)]

F32 = mybir.dt.float32
F32R = mybir.dt.float32r
BF16 = mybir.dt.bfloat16
ALU = mybir.AluOpType

MODE = "f32r_o4b2"

_cache = {}


def _rne(a: np.ndarray, mant_bits: int = 11) -> np.ndarray:
    """Round fp32 array to mant_bits mantissa bits, round-to-nearest-even."""
    drop = 23 - mant_bits
    u = np.ascontiguousarray(a, dtype=np.float32).view(np.uint32)
    lsb = (u >> drop) & 1
    u = u + ((1 << (drop - 1)) - 1) + lsb
    u &= np.uint32(0xFFFFFFFF) ^ np.uint32((1 << drop) - 1)
    return u.view(np.float32)


def _build_nc():
    nc = bacc.Bacc(trn_type="TRN2", target_bir_lowering=False)

    # DRAM I/O (per core). x_packed[k, blk] is a contiguous [128, COLS]
    # tile: host-transposed, column-prescaled by s_tau, RNE-rounded.
    x_d = nc.dram_tensor("x_packed", [KT, N_BLK, 128, COLS], F32R,
                         kind="ExternalInput")
    w_d = nc.dram_tensor("w_packed", [128, KT, N_CHUNK, 128], F32R,
                         kind="ExternalInput")
    wb_d = nc.dram_tensor("wb", [1, N_CHUNK, 128], F32R, kind="ExternalInput")
    xb_d = nc.dram_tensor("xb", [1, COLS], F32R, kind="ExternalInput")
    out_d = nc.dram_tensor("out", [128, T, N_CHUNK, B_LOC], BF16,
                           kind="ExternalOutput")

    with tile.TileContext(nc) as tc:
        with (
            tc.tile_pool(name="wpool", bufs=1) as wpool,
            tc.tile_pool(name="xpool", bufs=6) as xpool,
            tc.tile_pool(name="state", bufs=1) as state,
            tc.tile_pool(name="spkpool", bufs=4) as spkpool,
            tc.tile_pool(name="psum", bufs=2, space="PSUM") as psum_pool,
        ):
            # Resident weights: per-k DMAs so the first matmuls can start
            # before the whole 4MB lands. gpsimd queue, parallel to x on
            # sync/scalar queues.
            w_all = wpool.tile([128, KT, N_CHUNK, 128], F32R)
            for k in range(KT):
                nc.gpsimd.dma_start(w_all[:, k], w_d[:, k])
            wb_t = wpool.tile([1, N_CHUNK, 128], F32R)
            nc.gpsimd.dma_start(wb_t[:], wb_d[:])
            xb_t = wpool.tile([1, COLS], F32R)
            nc.gpsimd.dma_start(xb_t[:], xb_d[:])

            # State: ybar = -d^-tau (mem - 1); mem_0 = 0 -> ybar = 1.
            ybar = state.tile([128, N_CHUNK, B_LOC], F32)
            nc.vector.memset(ybar[:], 1.0)

            for bi in range(N_BLK):
                ps = psum_pool.tile([128, N_CHUNK, COLS], F32, tag="ps",
                                    name=f"ps_{bi}")
                for k in range(KT):
                    xt = xpool.tile([128, COLS], F32R, tag="xt",
                                    name=f"xt_{bi}_{k}")
                    eng = nc.sync if k % 2 == 0 else nc.scalar
                    eng.dma_start(xt[:], x_d[k, bi])
                    for c in range(N_CHUNK):
                        # start=True clears has_written for the WHOLE bank,
                        # so when chunks share a bank only the first chunk
                        # in the bank may clear; the others overwrite onto
                        # cleared bits.
                        nc.tensor.matmul(
                            ps[:, c, :], w_all[:, k, c, :], xt[:],
                            start=(k == 0 and c % CHUNKS_PER_BANK == 0),
                            stop=False,
                        )
                # Rank-1 bias row closes each chunk's accumulation group:
                # adds s_tau * (b_o + d - 1) to every column.
                for c in range(N_CHUNK):
                    nc.tensor.matmul(
                        ps[:, c, :], wb_t[:, c, :], xb_t[:],
                        start=False, stop=True,
                    )

                # LIF scan consuming this block's psum
                for tau in range(STEPS_PER_BLK):
                    t = bi * STEPS_PER_BLK + tau
                    g = ps[:, :, tau * B_LOC:(tau + 1) * B_LOC]
                    nc.vector.tensor_tensor(ybar[:], ybar[:], g,
                                            op=ALU.subtract)
                    spk = spkpool.tile([128, N_CHUNK, B_LOC], BF16, tag="spk")
                    nc.vector.tensor_scalar(spk[:], ybar[:], 0.0, None,
                                            op0=ALU.is_lt)
                    nc.vector.scalar_tensor_tensor(
                        ybar[:], spk[:], SCALES[tau], ybar[:],
                        op0=ALU.mult, op1=ALU.add,
                    )
                    nc.gpsimd.dma_start(out_d[:, t], spk[:])
                if bi + 1 < N_BLK:
                    nc.vector.tensor_scalar_mul(ybar[:], ybar[:],
                                                DECAY ** STEPS_PER_BLK)

    nc.compile()
    return nc


def _get_nc():
    if "nc" not in _cache:
        _cache["nc"] = _build_nc()
    return _cache["nc"]


def kernel(x_seq: np.ndarray, W: np.ndarray, b: np.ndarray) -> np.ndarray:
    nc = _get_nc()

    x_seq = np.ascontiguousarray(x_seq, dtype=np.float32)
    col_scale = np.array([SCALES[t % STEPS_PER_BLK] for t in range(T)],
                         dtype=np.float32)

    # Per-batch-shard x: [KT, N_BLK, 128, COLS], prescaled + RNE'd.
    x_parts = []
    for bc in range(BC_SHARD):
        xs = x_seq[:, bc * B_LOC:(bc + 1) * B_LOC, :]      # [T, B_LOC, I]
        xs = xs * col_scale[:, None, None]
        xp = xs.transpose(2, 0, 1)                         # [I, T, B_LOC]
        xp = xp.reshape(KT, 128, N_BLK, STEPS_PER_BLK * B_LOC)
        xp = np.ascontiguousarray(xp.transpose(0, 2, 1, 3))
        x_parts.append(_rne(xp))

    # Per-out-shard weights: [128(ip), KT, N_CHUNK, 128(of)], RNE'd.
    w_parts, wb_parts = [], []
    for oc in range(OC_SHARD):
        w_oc = W[oc * O_LOC:(oc + 1) * O_LOC, :].astype(np.float32)
        wT = _rne(w_oc.T)                                  # [I, O_LOC]
        wp = wT.reshape(KT, 128, N_CHUNK, 128).transpose(1, 0, 2, 3)
        w_parts.append(np.ascontiguousarray(wp))
        wb = b[oc * O_LOC:(oc + 1) * O_LOC].astype(np.float32) + DECAY - 1.0
        wb_parts.append(_rne(wb.reshape(1, N_CHUNK, 128)))

    xb = np.repeat(np.array(SCALES, dtype=np.float32), B_LOC).reshape(1, COLS)
    xb = _rne(xb)

    in_maps = []
    for c in range(N_CORES):
        oc, bc = divmod(c, BC_SHARD)
        in_maps.append({
            "x_packed": x_parts[bc],
            "w_packed": w_parts[oc],
            "wb": wb_parts[oc],
            "xb": xb,
        })

    res = bass_utils.run_bass_kernel_spmd(nc, in_maps, core_ids=list(range(N_CORES)))
    global LAST_RESULT
    LAST_RESULT = res

    # Assemble: out_c[op, t, chunk, beta] (bf16 {0,1}) -> [t, b, o] fp32
    result = np.empty((T, B, O), dtype=np.float32)
    for c in range(N_CORES):
        oc, bc = divmod(c, BC_SHARD)
        o_part = res.results[c]["out"].astype(np.float32)  # [128, T, 4, 64]
        part = o_part.transpose(1, 3, 2, 0).reshape(T, B_LOC, O_LOC)
        result[:, bc * B_LOC:(bc + 1) * B_LOC,
               oc * O_LOC:(oc + 1) * O_LOC] = part
    return result


LAST_RESULT = None


# revision 9
# speedup vs baseline: 1.0719x; 1.0719x over previous
"""LIF layer (T=64, B=128, 2048->2048) on 8 trn2 NeuronCores.

Sharding: 4-way over out_dim x 2-way over batch. Each core owns
O_loc=512 output channels (4 chunks of 128) and B_loc=64 batch rows.
The 512-wide moving streams let the f32r weight load (~195ns, no FWL
for 4-byte weights) hide behind each 213ns matmul, keeping the PE at
the f32r roofline (~116us/core).

GEMM: single-pass float32r (PE rounds inputs to ~fp22 with RNE;
1 cycle/row). Host pre-rounds inputs RNE to 11 mantissa bits (measured
bit-identical to HW rounding -> documents the precision contract).
Per 8-timestep block, psum holds [128, 4 chunks, 512 cols]; chunk <->
bank, double-buffered across blocks.

Scan: z-space reformulation removes the per-step decay multiply. Host
prescales x columns of step tau (within a block) by s_tau = d^-(tau+1);
bias and the threshold shift enter through a rank-1 17th matmul (bias
row). With the sign-flipped state ybar = -d^-tau*(mem-1), one LIF step
is 3 DVE ops:
    A: ybar -= G               (psum read)
    B: spk = ybar < 0          (exact {0,1}, written bf16 for cheap DMA)
    C: ybar = spk*s_tau + ybar (fused scalar_tensor_tensor)
and ybar *= d^8 once per block. Spikes DMA out per step; host casts
bf16 {0,1} back to fp32 exactly.
"""

import math

import numpy as np

import concourse.bacc as bacc
import concourse.bass as bass
import concourse.mybir as mybir
import concourse.tile as tile
from concourse import bass_utils

# Problem constants (hardcoded per contract)
T, B, I, O = 64, 128, 2048, 2048
N_CORES = 8
OC_SHARD, BC_SHARD = 2, 4          # out_dim x batch sharding grid
O_LOC = O // OC_SHARD              # 512 channels per core
B_LOC = B // BC_SHARD              # 64 batch rows per core
N_CHUNK = O_LOC // 128             # 4 stationary chunks
KT = I // 128                      # 16 k-tiles
STEPS_PER_BLK = 8                  # timesteps per psum block
N_BLK = T // STEPS_PER_BLK         # 8 blocks
COLS = STEPS_PER_BLK * B_LOC       # 512 moving columns per block
CHUNKS_PER_BANK = max(1, 512 // COLS)
TAU_C, THR = 2.0, 1.0
DECAY = math.exp(-1.0 / TAU_C)
SCALES = [DECAY ** -(t + 1) for t in range(STEPS_PER_BLK)]

F32 = mybir.dt.float32
F32R = mybir.dt.float32r
BF16 = mybir.dt.bfloat16
ALU = mybir.AluOpType

MODE = "f32r_o2b4v4"

_cache = {}


def _rne(a: np.ndarray, mant_bits: int = 11) -> np.ndarray:
    """Round fp32 array to mant_bits mantissa bits, round-to-nearest-even."""
    drop = 23 - mant_bits
    u = np.ascontiguousarray(a, dtype=np.float32).view(np.uint32)
    lsb = (u >> drop) & 1
    u = u + ((1 << (drop - 1)) - 1) + lsb
    u &= np.uint32(0xFFFFFFFF) ^ np.uint32((1 << drop) - 1)
    return u.view(np.float32)


def _build_nc():
    nc = bacc.Bacc(trn_type="TRN2", target_bir_lowering=False)

    # DRAM I/O (per core). x_packed[k, blk] is a contiguous [128, COLS]
    # tile: host-transposed, column-prescaled by s_tau, RNE-rounded.
    x_d = nc.dram_tensor("x_packed", [KT, N_BLK, 128, COLS], F32R,
                         kind="ExternalInput")
    w_d = nc.dram_tensor("w_packed", [128, KT, N_CHUNK, 128], F32R,
                         kind="ExternalInput")
    wb_d = nc.dram_tensor("wb", [1, N_CHUNK, 128], F32R, kind="ExternalInput")
    xb_d = nc.dram_tensor("xb", [1, COLS], F32R, kind="ExternalInput")
    out_d = nc.dram_tensor("out", [128, T, N_CHUNK, B_LOC], BF16,
                           kind="ExternalOutput")

    with tile.TileContext(nc) as tc:
        with (
            tc.tile_pool(name="wpool", bufs=1) as wpool,
            tc.tile_pool(name="xpool", bufs=6) as xpool,
            tc.tile_pool(name="state", bufs=1) as state,
            tc.tile_pool(name="spkpool", bufs=4) as spkpool,
            tc.tile_pool(name="psum", bufs=2, space="PSUM") as psum_pool,
        ):
            # Resident weights: per-k DMAs so the first matmuls can start
            # before the whole 4MB lands. gpsimd queue, parallel to x on
            # sync/scalar queues.
            wb_t = wpool.tile([1, N_CHUNK, 128], F32R)
            nc.gpsimd.dma_start(wb_t[:], wb_d[:])
            xb_t = wpool.tile([1, COLS], F32R)
            nc.gpsimd.dma_start(xb_t[:], xb_d[:])
            w_all = wpool.tile([128, KT, N_CHUNK, 128], F32R)
            for k in range(KT):
                nc.gpsimd.dma_start(w_all[:, k], w_d[:, k])

            # State: ybar = -d^-tau (mem - 1); mem_0 = 0 -> ybar = 1.
            ybar = state.tile([128, N_CHUNK, B_LOC], F32)
            nc.vector.memset(ybar[:], 1.0)

            # Pre-warm the PE HAM clock gate while the big W DMA streams:
            # ~12 garbage rank-1 matmuls into the first psum buffer (the
            # real k=0 start=True clears has_written over it afterwards).
            warm = psum_pool.tile([128, N_CHUNK, COLS], F32, tag="ps",
                                  name="ps_warm")
            for i in range(12):
                nc.tensor.matmul(warm[:, 0, :], wb_t[:, 0, :], xb_t[:],
                                 start=True, stop=(i == 11))

            for bi in range(N_BLK):
                ps = psum_pool.tile([128, N_CHUNK, COLS], F32, tag="ps",
                                    name=f"ps_{bi}")
                for k in range(KT):
                    xt = xpool.tile([128, COLS], F32R, tag="xt",
                                    name=f"xt_{bi}_{k}")
                    eng = nc.sync if k % 2 == 0 else nc.scalar
                    eng.dma_start(xt[:], x_d[k, bi])
                    for c in range(N_CHUNK):
                        # start=True clears has_written for the WHOLE bank,
                        # so when chunks share a bank only the first chunk
                        # in the bank may clear; the others overwrite onto
                        # cleared bits.
                        nc.tensor.matmul(
                            ps[:, c, :], w_all[:, k, c, :], xt[:],
                            start=(k == 0 and c % CHUNKS_PER_BANK == 0),
                            stop=False,
                        )
                # Rank-1 bias row closes each chunk's accumulation group:
                # adds s_tau * (b_o + d - 1) to every column.
                for c in range(N_CHUNK):
                    nc.tensor.matmul(
                        ps[:, c, :], wb_t[:, c, :], xb_t[:],
                        start=False, stop=True,
                    )

                # LIF scan consuming this block's psum
                for tau in range(STEPS_PER_BLK):
                    t = bi * STEPS_PER_BLK + tau
                    g = ps[:, :, tau * B_LOC:(tau + 1) * B_LOC]
                    nc.vector.tensor_tensor(ybar[:], ybar[:], g,
                                            op=ALU.subtract)
                    spk = spkpool.tile([128, N_CHUNK, B_LOC], BF16, tag="spk")
                    nc.vector.tensor_scalar(spk[:], ybar[:], 0.0, None,
                                            op0=ALU.is_lt)
                    nc.vector.scalar_tensor_tensor(
                        ybar[:], spk[:], SCALES[tau], ybar[:],
                        op0=ALU.mult, op1=ALU.add,
                    )
                    nc.gpsimd.dma_start(out_d[:, t], spk[:])
                if bi + 1 < N_BLK:
                    nc.vector.tensor_scalar_mul(ybar[:], ybar[:],
                                                DECAY ** STEPS_PER_BLK)

    nc.compile()
    return nc


def _get_nc():
    if "nc" not in _cache:
        _cache["nc"] = _build_nc()
    return _cache["nc"]


def kernel(x_seq: np.ndarray, W: np.ndarray, b: np.ndarray) -> np.ndarray:
    nc = _get_nc()

    x_seq = np.ascontiguousarray(x_seq, dtype=np.float32)
    col_scale = np.array([SCALES[t % STEPS_PER_BLK] for t in range(T)],
                         dtype=np.float32)

    # Per-batch-shard x: [KT, N_BLK, 128, COLS], prescaled + RNE'd.
    x_parts = []
    for bc in range(BC_SHARD):
        xs = x_seq[:, bc * B_LOC:(bc + 1) * B_LOC, :]      # [T, B_LOC, I]
        xs = xs * col_scale[:, None, None]
        xp = xs.transpose(2, 0, 1)                         # [I, T, B_LOC]
        xp = xp.reshape(KT, 128, N_BLK, STEPS_PER_BLK * B_LOC)
        xp = np.ascontiguousarray(xp.transpose(0, 2, 1, 3))
        x_parts.append(_rne(xp))

    # Per-out-shard weights: [128(ip), KT, N_CHUNK, 128(of)], RNE'd.
    w_parts, wb_parts = [], []
    for oc in range(OC_SHARD):
        w_oc = W[oc * O_LOC:(oc + 1) * O_LOC, :].astype(np.float32)
        wT = _rne(w_oc.T)                                  # [I, O_LOC]
        wp = wT.reshape(KT, 128, N_CHUNK, 128).transpose(1, 0, 2, 3)
        w_parts.append(np.ascontiguousarray(wp))
        wb = b[oc * O_LOC:(oc + 1) * O_LOC].astype(np.float32) + DECAY - 1.0
        wb_parts.append(_rne(wb.reshape(1, N_CHUNK, 128)))

    xb = np.repeat(np.array(SCALES, dtype=np.float32), B_LOC).reshape(1, COLS)
    xb = _rne(xb)

    in_maps = []
    for c in range(N_CORES):
        oc, bc = divmod(c, BC_SHARD)
        in_maps.append({
            "x_packed": x_parts[bc],
            "w_packed": w_parts[oc],
            "wb": wb_parts[oc],
            "xb": xb,
        })

    res = bass_utils.run_bass_kernel_spmd(nc, in_maps, core_ids=list(range(N_CORES)))
    global LAST_RESULT
    LAST_RESULT = res

    # Assemble: out_c[op, t, chunk, beta] (bf16 {0,1}) -> [t, b, o] fp32
    result = np.empty((T, B, O), dtype=np.float32)
    for c in range(N_CORES):
        oc, bc = divmod(c, BC_SHARD)
        o_part = res.results[c]["out"].astype(np.float32)  # [128, T, 4, 64]
        part = o_part.transpose(1, 3, 2, 0).reshape(T, B_LOC, O_LOC)
        result[:, bc * B_LOC:(bc + 1) * B_LOC,
               oc * O_LOC:(oc + 1) * O_LOC] = part
    return result


LAST_RESULT = None


# revision 12
# speedup vs baseline: 1.0942x; 1.0208x over previous
"""LIF layer (T=64, B=128, 2048->2048) on 8 trn2 NeuronCores.

Sharding: 4-way over out_dim x 2-way over batch. Each core owns
O_loc=512 output channels (4 chunks of 128) and B_loc=64 batch rows.
The 512-wide moving streams let the f32r weight load (~195ns, no FWL
for 4-byte weights) hide behind each 213ns matmul, keeping the PE at
the f32r roofline (~116us/core).

GEMM: single-pass float32r (PE rounds inputs to ~fp22 with RNE;
1 cycle/row). Host pre-rounds inputs RNE to 11 mantissa bits (measured
bit-identical to HW rounding -> documents the precision contract).
Per 8-timestep block, psum holds [128, 4 chunks, 512 cols]; chunk <->
bank, double-buffered across blocks.

Scan: z-space reformulation removes the per-step decay multiply. Host
prescales x columns of step tau (within a block) by s_tau = d^-(tau+1);
bias and the threshold shift enter through a rank-1 17th matmul (bias
row). With the sign-flipped state ybar = -d^-tau*(mem-1), one LIF step
is 3 DVE ops:
    A: ybar -= G               (psum read)
    B: spk = ybar < 0          (exact {0,1}, written bf16 for cheap DMA)
    C: ybar = spk*s_tau + ybar (fused scalar_tensor_tensor)
and ybar *= d^8 once per block. Spikes DMA out per step; host casts
bf16 {0,1} back to fp32 exactly.
"""

import math

import numpy as np

import concourse.bacc as bacc
import concourse.bass as bass
import concourse.mybir as mybir
import concourse.tile as tile
from concourse import bass_utils

# Problem constants (hardcoded per contract)
T, B, I, O = 64, 128, 2048, 2048
N_CORES = 8
OC_SHARD, BC_SHARD = 2, 4          # out_dim x batch sharding grid
O_LOC = O // OC_SHARD              # 512 channels per core
B_LOC = B // BC_SHARD              # 64 batch rows per core
N_CHUNK = O_LOC // 128             # 4 stationary chunks
KT = I // 128                      # 16 k-tiles
STEPS_PER_BLK = 8                  # timesteps per psum block
N_BLK = T // STEPS_PER_BLK         # 8 blocks
COLS = STEPS_PER_BLK * B_LOC       # 512 moving columns per block
CHUNKS_PER_BANK = max(1, 512 // COLS)
TAU_C, THR = 2.0, 1.0
DECAY = math.exp(-1.0 / TAU_C)
SCALES = [DECAY ** -(t + 1) for t in range(STEPS_PER_BLK)]

F32 = mybir.dt.float32
F32R = mybir.dt.float32r
BF16 = mybir.dt.bfloat16
ALU = mybir.AluOpType

MODE = "f32r_o2b4v5b"

_cache = {}


def _rne(a: np.ndarray, mant_bits: int = 11) -> np.ndarray:
    """Round fp32 array to mant_bits mantissa bits, round-to-nearest-even."""
    drop = 23 - mant_bits
    u = np.ascontiguousarray(a, dtype=np.float32).view(np.uint32)
    lsb = (u >> drop) & 1
    u = u + ((1 << (drop - 1)) - 1) + lsb
    u &= np.uint32(0xFFFFFFFF) ^ np.uint32((1 << drop) - 1)
    return u.view(np.float32)


def _build_nc():
    nc = bacc.Bacc(trn_type="TRN2", target_bir_lowering=False)

    # DRAM I/O (per core). x_packed[k, blk] is a contiguous [128, COLS]
    # tile: host-transposed, column-prescaled by s_tau, RNE-rounded.
    x_d = nc.dram_tensor("x_packed", [KT, N_BLK, 128, COLS], F32R,
                         kind="ExternalInput")
    w_d = nc.dram_tensor("w_packed", [128, KT, N_CHUNK, 128], F32R,
                         kind="ExternalInput")
    wb_d = nc.dram_tensor("wb", [1, N_CHUNK, 128], F32R, kind="ExternalInput")
    xb_d = nc.dram_tensor("xb", [1, COLS], F32R, kind="ExternalInput")
    out_d = nc.dram_tensor("out", [128, T, N_CHUNK, B_LOC], BF16,
                           kind="ExternalOutput")

    with tile.TileContext(nc) as tc:
        with (
            tc.tile_pool(name="wpool", bufs=1) as wpool,
            tc.tile_pool(name="xpool", bufs=6) as xpool,
            tc.tile_pool(name="state", bufs=1) as state,
            tc.tile_pool(name="spkpool", bufs=4) as spkpool,
            tc.tile_pool(name="psum", bufs=2, space="PSUM") as psum_pool,
        ):
            # Resident weights: per-k DMAs so the first matmuls can start
            # before the whole 4MB lands. gpsimd queue, parallel to x on
            # sync/scalar queues.
            # Warmup operands come from memsets (no DMA dependency), so
            # the PE HAM pre-warm can start within ~1us of kernel entry.
            wsrc = state.tile([1, 128], F32)
            nc.vector.memset(wsrc[:], 0.0)
            xsrc = state.tile([1, COLS], F32)
            nc.vector.memset(xsrc[:], 0.0)

            # W streams on the sync+scalar rings, interleaved k-wise with
            # block-0 x tiles (issued in the bi==0 loop below) so both stay
            # just ahead of GEMM consumption. gpsimd's ring starts ~10us
            # late (global dma_reset/sem_clear preamble), so it only gets
            # the small bias tiles and the spike writeback.
            w_all = wpool.tile([128, KT, N_CHUNK, 128], F32R)
            wb_t = wpool.tile([1, N_CHUNK, 128], F32R)
            nc.gpsimd.dma_start(wb_t[:], wb_d[:])
            xb_t = wpool.tile([1, COLS], F32R)
            nc.gpsimd.dma_start(xb_t[:], xb_d[:])

            # State: ybar = -d^-tau (mem - 1); mem_0 = 0 -> ybar = 1.
            ybar = state.tile([128, N_CHUNK, B_LOC], F32)
            nc.vector.memset(ybar[:], 1.0)

            # Pre-warm the PE HAM clock gate while the W DMA streams:
            # garbage rank-1 matmuls into the first psum buffer (the
            # real k=0 start=True clears has_written over it afterwards).
            warm = psum_pool.tile([128, N_CHUNK, COLS], F32, tag="ps",
                                  name="ps_warm")
            for i in range(16):
                nc.tensor.matmul(warm[:, 0, :], wsrc[:], xsrc[:],
                                 start=True, stop=(i == 15))

            for bi in range(N_BLK):
                ps = psum_pool.tile([128, N_CHUNK, COLS], F32, tag="ps",
                                    name=f"ps_{bi}")
                for k in range(KT):
                    xt = xpool.tile([128, COLS], F32R, tag="xt",
                                    name=f"xt_{bi}_{k}")
                    eng = nc.sync if k % 2 == 0 else nc.scalar
                    eng.dma_start(xt[:], x_d[k, bi])
                    if bi == 0:
                        eng.dma_start(w_all[:, k], w_d[:, k])
                    for c in range(N_CHUNK):
                        # start=True clears has_written for the WHOLE bank,
                        # so when chunks share a bank only the first chunk
                        # in the bank may clear; the others overwrite onto
                        # cleared bits.
                        nc.tensor.matmul(
                            ps[:, c, :], w_all[:, k, c, :], xt[:],
                            start=(k == 0 and c % CHUNKS_PER_BANK == 0),
                            stop=False,
                        )
                # Rank-1 bias row closes each chunk's accumulation group:
                # adds s_tau * (b_o + d - 1) to every column.
                for c in range(N_CHUNK):
                    nc.tensor.matmul(
                        ps[:, c, :], wb_t[:, c, :], xb_t[:],
                        start=False, stop=True,
                    )

                # LIF scan consuming this block's psum
                for tau in range(STEPS_PER_BLK):
                    t = bi * STEPS_PER_BLK + tau
                    g = ps[:, :, tau * B_LOC:(tau + 1) * B_LOC]
                    nc.vector.tensor_tensor(ybar[:], ybar[:], g,
                                            op=ALU.subtract)
                    spk = spkpool.tile([128, N_CHUNK, B_LOC], BF16, tag="spk")
                    nc.vector.tensor_scalar(spk[:], ybar[:], 0.0, None,
                                            op0=ALU.is_lt)
                    if t + 1 < T:
                        nc.vector.scalar_tensor_tensor(
                            ybar[:], spk[:], SCALES[tau], ybar[:],
                            op0=ALU.mult, op1=ALU.add,
                        )
                    nc.gpsimd.dma_start(out_d[:, t], spk[:])
                if bi + 1 < N_BLK:
                    nc.vector.tensor_scalar_mul(ybar[:], ybar[:],
                                                DECAY ** STEPS_PER_BLK)

    nc.compile()
    return nc


def _get_nc():
    if "nc" not in _cache:
        _cache["nc"] = _build_nc()
    return _cache["nc"]


def kernel(x_seq: np.ndarray, W: np.ndarray, b: np.ndarray) -> np.ndarray:
    nc = _get_nc()

    x_seq = np.ascontiguousarray(x_seq, dtype=np.float32)
    col_scale = np.array([SCALES[t % STEPS_PER_BLK] for t in range(T)],
                         dtype=np.float32)

    # Per-batch-shard x: [KT, N_BLK, 128, COLS], prescaled + RNE'd.
    x_parts = []
    for bc in range(BC_SHARD):
        xs = x_seq[:, bc * B_LOC:(bc + 1) * B_LOC, :]      # [T, B_LOC, I]
        xs = xs * col_scale[:, None, None]
        xp = xs.transpose(2, 0, 1)                         # [I, T, B_LOC]
        xp = xp.reshape(KT, 128, N_BLK, STEPS_PER_BLK * B_LOC)
        xp = np.ascontiguousarray(xp.transpose(0, 2, 1, 3))
        x_parts.append(_rne(xp))

    # Per-out-shard weights: [128(ip), KT, N_CHUNK, 128(of)], RNE'd.
    w_parts, wb_parts = [], []
    for oc in range(OC_SHARD):
        w_oc = W[oc * O_LOC:(oc + 1) * O_LOC, :].astype(np.float32)
        wT = _rne(w_oc.T)                                  # [I, O_LOC]
        wp = wT.reshape(KT, 128, N_CHUNK, 128).transpose(1, 0, 2, 3)
        w_parts.append(np.ascontiguousarray(wp))
        wb = b[oc * O_LOC:(oc + 1) * O_LOC].astype(np.float32) + DECAY - 1.0
        wb_parts.append(_rne(wb.reshape(1, N_CHUNK, 128)))

    xb = np.repeat(np.array(SCALES, dtype=np.float32), B_LOC).reshape(1, COLS)
    xb = _rne(xb)

    in_maps = []
    for c in range(N_CORES):
        oc, bc = divmod(c, BC_SHARD)
        in_maps.append({
            "x_packed": x_parts[bc],
            "w_packed": w_parts[oc],
            "wb": wb_parts[oc],
            "xb": xb,
        })

    res = bass_utils.run_bass_kernel_spmd(nc, in_maps, core_ids=list(range(N_CORES)))
    global LAST_RESULT
    LAST_RESULT = res

    # Assemble: out_c[op, t, chunk, beta] (bf16 {0,1}) -> [t, b, o] fp32
    result = np.empty((T, B, O), dtype=np.float32)
    for c in range(N_CORES):
        oc, bc = divmod(c, BC_SHARD)
        o_part = res.results[c]["out"].astype(np.float32)  # [128, T, 4, 64]
        part = o_part.transpose(1, 3, 2, 0).reshape(T, B_LOC, O_LOC)
        result[:, bc * B_LOC:(bc + 1) * B_LOC,
               oc * O_LOC:(oc + 1) * O_LOC] = part
    return result


LAST_RESULT = None


# revision 13
# speedup vs baseline: 1.1320x; 1.0345x over previous
"""LIF layer (T=64, B=128, 2048->2048) on 8 trn2 NeuronCores.

Sharding: 4-way over out_dim x 2-way over batch. Each core owns
O_loc=512 output channels (4 chunks of 128) and B_loc=64 batch rows.
The 512-wide moving streams let the f32r weight load (~195ns, no FWL
for 4-byte weights) hide behind each 213ns matmul, keeping the PE at
the f32r roofline (~116us/core).

GEMM: single-pass float32r (PE rounds inputs to ~fp22 with RNE;
1 cycle/row). Host pre-rounds inputs RNE to 11 mantissa bits (measured
bit-identical to HW rounding -> documents the precision contract).
Per 8-timestep block, psum holds [128, 4 chunks, 512 cols]; chunk <->
bank, double-buffered across blocks.

Scan: z-space reformulation removes the per-step decay multiply. Host
prescales x columns of step tau (within a block) by s_tau = d^-(tau+1);
bias and the threshold shift enter through a rank-1 17th matmul (bias
row). With the sign-flipped state ybar = -d^-tau*(mem-1), one LIF step
is 3 DVE ops:
    A: ybar -= G               (psum read)
    B: spk = ybar < 0          (exact {0,1}, written bf16 for cheap DMA)
    C: ybar = spk*s_tau + ybar (fused scalar_tensor_tensor)
and ybar *= d^8 once per block. Spikes DMA out per step; host casts
bf16 {0,1} back to fp32 exactly.
"""

import math

import numpy as np

import concourse.bacc as bacc
import concourse.bass as bass
import concourse.mybir as mybir
import concourse.tile as tile
from concourse import bass_utils

# Problem constants (hardcoded per contract)
T, B, I, O = 64, 128, 2048, 2048
N_CORES = 8
OC_SHARD, BC_SHARD = 2, 4          # out_dim x batch sharding grid
O_LOC = O // OC_SHARD              # 512 channels per core
B_LOC = B // BC_SHARD              # 64 batch rows per core
N_CHUNK = O_LOC // 128             # 4 stationary chunks
KT = I // 128                      # 16 k-tiles
STEPS_PER_BLK = 8                  # timesteps per psum block
N_BLK = T // STEPS_PER_BLK         # 8 blocks
COLS = STEPS_PER_BLK * B_LOC       # 512 moving columns per block
CHUNKS_PER_BANK = max(1, 512 // COLS)
TAU_C, THR = 2.0, 1.0
DECAY = math.exp(-1.0 / TAU_C)
SCALES = [DECAY ** -(t + 1) for t in range(STEPS_PER_BLK)]

F32 = mybir.dt.float32
F32R = mybir.dt.float32r
BF16 = mybir.dt.bfloat16
ALU = mybir.AluOpType

MODE = "f32r_o2b4v6"

_cache = {}


def _rne(a: np.ndarray, mant_bits: int = 11) -> np.ndarray:
    """Round fp32 array to mant_bits mantissa bits, round-to-nearest-even."""
    drop = 23 - mant_bits
    u = np.ascontiguousarray(a, dtype=np.float32).view(np.uint32)
    lsb = (u >> drop) & 1
    u = u + ((1 << (drop - 1)) - 1) + lsb
    u &= np.uint32(0xFFFFFFFF) ^ np.uint32((1 << drop) - 1)
    return u.view(np.float32)


def _build_nc():
    nc = bacc.Bacc(trn_type="TRN2", target_bir_lowering=False)

    # DRAM I/O (per core). x_packed[k, blk] is a contiguous [128, COLS]
    # tile: host-transposed, column-prescaled by s_tau, RNE-rounded.
    x_d = nc.dram_tensor("x_packed", [KT, N_BLK, 128, COLS], F32R,
                         kind="ExternalInput")
    w_d = nc.dram_tensor("w_packed", [128, KT, N_CHUNK, 128], F32R,
                         kind="ExternalInput")
    wb_d = nc.dram_tensor("wb", [1, N_CHUNK, 128], F32R, kind="ExternalInput")
    xb_d = nc.dram_tensor("xb", [1, COLS], F32R, kind="ExternalInput")
    out_d = nc.dram_tensor("out", [128, T, N_CHUNK, B_LOC], BF16,
                           kind="ExternalOutput")

    with tile.TileContext(nc) as tc:
        with (
            tc.tile_pool(name="wpool", bufs=1) as wpool,
            tc.tile_pool(name="xpool", bufs=6) as xpool,
            tc.tile_pool(name="state", bufs=1) as state,
            tc.tile_pool(name="spkpool", bufs=4) as spkpool,
            tc.tile_pool(name="psum", bufs=2, space="PSUM") as psum_pool,
        ):
            # Resident weights: per-k DMAs so the first matmuls can start
            # before the whole 4MB lands. gpsimd queue, parallel to x on
            # sync/scalar queues.
            # All bulk DMAs ride the sync+scalar HWDGE rings; gpsimd's
            # ring is software-DGE (slow ~6us drain) and its first DMA is
            # delayed ~10us by the global dma_reset/sem_clear preamble, so
            # it carries nothing. W streams interleaved k-wise with
            # block-0 x tiles (issued in the bi==0 loop below) so both
            # stay just ahead of GEMM consumption.
            w_all = wpool.tile([128, KT, N_CHUNK, 128], F32R)
            wb_t = wpool.tile([1, N_CHUNK, 128], F32R)
            nc.sync.dma_start(wb_t[:], wb_d[:])
            xb_t = wpool.tile([1, COLS], F32R)
            nc.scalar.dma_start(xb_t[:], xb_d[:])

            # State: ybar = -d^-tau (mem - 1); mem_0 = 0 -> ybar = 1.
            ybar = state.tile([128, N_CHUNK, B_LOC], F32)
            nc.vector.memset(ybar[:], 1.0)

            for bi in range(N_BLK):
                ps = psum_pool.tile([128, N_CHUNK, COLS], F32, tag="ps",
                                    name=f"ps_{bi}")
                for k in range(KT):
                    xt = xpool.tile([128, COLS], F32R, tag="xt",
                                    name=f"xt_{bi}_{k}")
                    eng = nc.sync if k % 2 == 0 else nc.scalar
                    eng.dma_start(xt[:], x_d[k, bi])
                    if bi == 0:
                        eng.dma_start(w_all[:, k], w_d[:, k])
                    for c in range(N_CHUNK):
                        # start=True clears has_written for the WHOLE bank,
                        # so when chunks share a bank only the first chunk
                        # in the bank may clear; the others overwrite onto
                        # cleared bits.
                        nc.tensor.matmul(
                            ps[:, c, :], w_all[:, k, c, :], xt[:],
                            start=(k == 0 and c % CHUNKS_PER_BANK == 0),
                            stop=False,
                        )
                # Rank-1 bias row closes each chunk's accumulation group:
                # adds s_tau * (b_o + d - 1) to every column.
                for c in range(N_CHUNK):
                    nc.tensor.matmul(
                        ps[:, c, :], wb_t[:, c, :], xb_t[:],
                        start=False, stop=True,
                    )

                # LIF scan consuming this block's psum. Spikes for two
                # consecutive steps share one tile and go out in a single
                # DMA (out[:, t-1:t+1] is contiguous), halving descriptors.
                for tau in range(STEPS_PER_BLK):
                    t = bi * STEPS_PER_BLK + tau
                    g = ps[:, :, tau * B_LOC:(tau + 1) * B_LOC]
                    nc.vector.tensor_tensor(ybar[:], ybar[:], g,
                                            op=ALU.subtract)
                    if tau % 2 == 0:
                        spk2 = spkpool.tile([128, 2, N_CHUNK, B_LOC], BF16,
                                            tag="spk")
                    spk = spk2[:, tau % 2]
                    nc.vector.tensor_scalar(spk, ybar[:], 0.0, None,
                                            op0=ALU.is_lt)
                    if t + 1 < T:
                        nc.vector.scalar_tensor_tensor(
                            ybar[:], spk, SCALES[tau], ybar[:],
                            op0=ALU.mult, op1=ALU.add,
                        )
                    if tau % 2 == 1:
                        eng = nc.sync if (t // 2) % 2 == 0 else nc.scalar
                        eng.dma_start(out_d[:, t - 1:t + 1], spk2[:])
                if bi + 1 < N_BLK:
                    nc.vector.tensor_scalar_mul(ybar[:], ybar[:],
                                                DECAY ** STEPS_PER_BLK)

    nc.compile()
    return nc


def _get_nc():
    if "nc" not in _cache:
        _cache["nc"] = _build_nc()
    return _cache["nc"]


def kernel(x_seq: np.ndarray, W: np.ndarray, b: np.ndarray) -> np.ndarray:
    nc = _get_nc()

    x_seq = np.ascontiguousarray(x_seq, dtype=np.float32)
    col_scale = np.array([SCALES[t % STEPS_PER_BLK] for t in range(T)],
                         dtype=np.float32)

    # Per-batch-shard x: [KT, N_BLK, 128, COLS], prescaled + RNE'd.
    x_parts = []
    for bc in range(BC_SHARD):
        xs = x_seq[:, bc * B_LOC:(bc + 1) * B_LOC, :]      # [T, B_LOC, I]
        xs = xs * col_scale[:, None, None]
        xp = xs.transpose(2, 0, 1)                         # [I, T, B_LOC]
        xp = xp.reshape(KT, 128, N_BLK, STEPS_PER_BLK * B_LOC)
        xp = np.ascontiguousarray(xp.transpose(0, 2, 1, 3))
        x_parts.append(_rne(xp))

    # Per-out-shard weights: [128(ip), KT, N_CHUNK, 128(of)], RNE'd.
    w_parts, wb_parts = [], []
    for oc in range(OC_SHARD):
        w_oc = W[oc * O_LOC:(oc + 1) * O_LOC, :].astype(np.float32)
        wT = _rne(w_oc.T)                                  # [I, O_LOC]
        wp = wT.reshape(KT, 128, N_CHUNK, 128).transpose(1, 0, 2, 3)
        w_parts.append(np.ascontiguousarray(wp))
        wb = b[oc * O_LOC:(oc + 1) * O_LOC].astype(np.float32) + DECAY - 1.0
        wb_parts.append(_rne(wb.reshape(1, N_CHUNK, 128)))

    xb = np.repeat(np.array(SCALES, dtype=np.float32), B_LOC).reshape(1, COLS)
    xb = _rne(xb)

    in_maps = []
    for c in range(N_CORES):
        oc, bc = divmod(c, BC_SHARD)
        in_maps.append({
            "x_packed": x_parts[bc],
            "w_packed": w_parts[oc],
            "wb": wb_parts[oc],
            "xb": xb,
        })

    res = bass_utils.run_bass_kernel_spmd(nc, in_maps, core_ids=list(range(N_CORES)))
    global LAST_RESULT
    LAST_RESULT = res

    # Assemble: out_c[op, t, chunk, beta] (bf16 {0,1}) -> [t, b, o] fp32
    result = np.empty((T, B, O), dtype=np.float32)
    for c in range(N_CORES):
        oc, bc = divmod(c, BC_SHARD)
        o_part = res.results[c]["out"].astype(np.float32)  # [128, T, 4, 64]
        part = o_part.transpose(1, 3, 2, 0).reshape(T, B_LOC, O_LOC)
        result[:, bc * B_LOC:(bc + 1) * B_LOC,
               oc * O_LOC:(oc + 1) * O_LOC] = part
    return result


LAST_RESULT = None
